# revision 29
# baseline (speedup 1.0000x reference)
"""Trainium2 Bass kernel for CLSAggregator: 6-layer dense transformer encoder
(ALiBi attention + SwiGLU MLP), B=4, S=1024, D=768, H=16, FF=3072.

Sharding: tokens split (batch, seq-half) -> 8 cores, 512 tokens each.
Per layer each core computes LN1/QKV/attention(local queries vs full keys of
its batch element)/Wo/LN2/SwiGLU for its token slab; K,V are exchanged within
core pairs via AllGather. Weights are replicated and streamed from HBM.

ALiBi bias: the 16 pad rows of each 64-padded head block carry position-
augmented entries so the QK matmul itself produces scores+bias for every
(query, key-block) combination whose |i-j| sign is uniform; the remaining
128-query "kink" windows (one per straddling key block) get an explicit
slope-diagonal matmul on per-core masked distance data.

Self-contained: hardcodes all shapes; host side folds LN weights into the
projection weights and precomputes the augmentation/kink tensors.
"""
import math
import os

import numpy as np
import ml_dtypes

import concourse.bass as bass
import concourse.mybir as mybir
import concourse.tile as tile
from concourse import bacc
from concourse.bass_utils import run_bass_kernel_spmd
from concourse.masks import make_identity

F32 = mybir.dt.float32
F32R = mybir.dt.float32r
BF16 = mybir.dt.bfloat16
FP16 = mybir.dt.float16
F8 = mybir.dt.float8e4
AF = mybir.ActivationFunctionType
OP = mybir.AluOpType
PM_DR = mybir.MatmulPerfMode.DoubleRow
FSC = 128.0             # fp8 weight staging scale (2**7)

L, H, D, FF = 6, 16, 768, 3072
B, NSEQ = 4, 1023
S = NSEQ + 1            # 1024
HD = D // H             # 48
EPS = 1e-5
NCORES = 8
T = S // 2              # 512 tokens per core
KT = D // 128           # 6 feature k-tiles
TT = T // 128           # 4 token tiles
FT = FF // 128          # 24 ff tiles
QK_PAD = H * 64         # 1024 padded q (or k) rows
VW = H * 49             # 784 v cols incl per-head ones-aug column
NAUG = 16               # aug rows per head (8 pairs)

_DTMAP = {"bf16": BF16, "f32r": F32R, "fp16": FP16}
DT_A = _DTMAP[os.environ.get("KDT_A", "fp16")]   # attention operands / Wo / Wd / KV
DT_H = _DTMAP[os.environ.get("KDT_H", "fp16")]   # QKV & gate/up weights + acts
KP = 768                # packed K rows (48 per head, no pad)

_NC_CACHE = {}


def build_nc(use_bqk, use_bgu, l_run=L, bare=False, slopes=None):
    assert not use_bgu, "fp8 FFN path does not support folded LN2 biases"
    nc = bacc.Bacc("TRN2", target_bir_lowering=False, debug=False,
                   enable_asserts=True, num_devices=NCORES)

    # ---- I/O ----
    x0_d = nc.dram_tensor("x0", [T, D], F32, kind="ExternalInput")
    sid_d = nc.dram_tensor("sid", [128, H * 128], FP16, kind="ExternalInput")
    kink_d = nc.dram_tensor("kinkd", [128, 8 * 128], FP16, kind="ExternalInput")
    kaug_d = nc.dram_tensor("kaug", [H, NAUG, S], DT_A, kind="ExternalInput")
    qaug_d = nc.dram_tensor("qaug", [H, NAUG, T], DT_A, kind="ExternalInput")
    wqk_d = nc.dram_tensor("wqk", [L, D, QK_PAD + KP], DT_H, kind="ExternalInput")
    wv_d = nc.dram_tensor("wv", [L, D, VW], DT_H, kind="ExternalInput")
    bv_d = nc.dram_tensor("bv", [L, 1, VW], DT_H, kind="ExternalInput")
    wo_d = nc.dram_tensor("wo", [L, QK_PAD, D], DT_A, kind="ExternalInput")
    wgu_d = nc.dram_tensor("wgu", [L, D, 2 * FF], DT_H, kind="ExternalInput")
    wd_d = nc.dram_tensor("wd", [L, FF, D], DT_A, kind="ExternalInput")
    if use_bqk:
        bqk_d = nc.dram_tensor("bqk", [L, 1, 2 * QK_PAD], DT_H, kind="ExternalInput")
    if use_bgu:
        bg_d = nc.dram_tensor("bg", [L, 1, FF], DT_H, kind="ExternalInput")
        bu_d = nc.dram_tensor("bu", [L, 1, FF], DT_H, kind="ExternalInput")
    finw_d = nc.dram_tensor("finw", [1, D], F32, kind="ExternalInput")
    finb_d = nc.dram_tensor("finb", [1, D], F32, kind="ExternalInput")
    y_d = nc.dram_tensor("y", [1, D], F32, kind="ExternalOutput")

    KELEM = KP * T
    VELEM = T * VW

    if bare:
        # overhead-measurement baseline: zero compute, same I/O signature
        with tile.TileContext(nc) as tc:
            with tc.tile_pool(name="pb", bufs=1) as pb:
                yt = pb.tile([1, D], F32, tag="fy", name="fy")
                nc.vector.memset(yt[:], 0.0)
                nc.sync.dma_start(y_d.ap(), yt[:])
        nc.compile()
        return nc

    with tile.TileContext(nc) as tc:
        with (
            tc.tile_pool(name="p1", bufs=1) as p1,
            tc.tile_pool(name="p2", bufs=2) as p2,
            tc.tile_pool(name="p3", bufs=3) as p3,
            tc.tile_pool(name="p4", bufs=4) as p4,
            tc.tile_pool(name="psmm", bufs=8, space="PSUM") as psmm,
            tc.tile_pool(name="dram", bufs=2, space="DRAM") as dram,
        ):
            # ---- persistent tiles ----
            ident = p1.tile([128, 128], F32, tag="ident", name="ident")
            make_identity(nc, ident[:])
            ones_f = p1.tile([1, 128], F32, tag="ones_f", name="ones_f")
            nc.vector.memset(ones_f[:], 1.0)
            ones_h = p1.tile([1, 128], DT_H, tag="ones_h", name="ones_h")    # K=1 lhsT for v bias
            nc.vector.tensor_copy(ones_h[:], ones_f[0:1, 0:128])
            if use_bqk or use_bgu:
                ones_row = p1.tile([1, T], DT_H, tag="ones_row", name="ones_row")
                nc.scalar.copy(ones_row[0:1, 0:128], ones_f[:])
                nc.scalar.copy(ones_row[0:1, 128:256], ones_f[:])
                nc.scalar.copy(ones_row[0:1, 256:384], ones_f[:])
                nc.scalar.copy(ones_row[0:1, 384:512], ones_f[:])
            epst = p1.tile([128, 1], F32, tag="epst", name="epst")
            nc.vector.memset(epst[:], EPS)
            maskf = p1.tile([1, 64], F32, tag="maskf", name="maskf")
            nc.vector.memset(maskf[:], 0.0)
            nc.vector.memset(maskf[0:1, 0:48], 1.0)
            mask48 = p1.tile([1, 64], DT_A, tag="mask48", name="mask48")
            nc.vector.tensor_copy(mask48[:], maskf[:])

            # Pre-zero all PSUM banks so never-written pad regions read as
            # finite values (avoids NaN poisoning through 0-weight matmuls).
            zps = [psmm.tile([128, 512], F32, tag="mm", name="mm")
                   for _ in range(8)]
            for z in zps:
                nc.vector.memset(z[:], 0.0)

            x = [p1.tile([128, D], F32, tag=f"x{t}", name=f"x{t}") for t in range(TT)]
            for t in range(TT):
                nc.sync.dma_start(x[t][:], x0_d.ap()[t * 128:(t + 1) * 128, :])

            sid = p1.tile([128, H * 128], FP16, tag="sid", name="sid")
            nc.sync.dma_start(sid[:], sid_d.ap())
            kink_sb = p1.tile([128, 8 * 128], FP16, tag="kink", name="kink")
            nc.sync.dma_start(kink_sb[:], kink_d.ap())

            # persistent attention tiles; ALiBi aug rows written once
            kT_full = [p1.tile([128, S], DT_A, tag=f"kTf{r}", name=f"kTf{r}")
                       for r in range(8)]
            for r in range(8):
                nc.sync.dma_start(kT_full[r][48:48 + NAUG, :], kaug_d.ap()[2 * r])
                nc.sync.dma_start(kT_full[r][112:112 + NAUG, :],
                                  kaug_d.ap()[2 * r + 1])
            # padded to 800 cols so 64-wide per-head lhsT slices stay in range
            v_full = [p1.tile([128, VW + 16], DT_A, tag=f"vf{j}", name=f"vf{j}")
                      for j in range(8)]
            for j in range(8):
                nc.vector.memset(v_full[j][:, VW:VW + 16], 0.0)
            qT = [p1.tile([128, T], DT_A, tag=f"qT{m}", name=f"qT{m}")
                  for m in range(8)]

            def layernorm_to(src_tiles, dst_fn):
                """LN over features (free dim of token-major src); transposed
                feature-major output written via dst_fn(d, t) -> AP."""
                for t in range(TT):
                    st = p2.tile([128, 12], F32, tag="bnst", name="bnst")
                    nc.vector.bn_stats(st[:, 0:6], src_tiles[t][:, 0:384])
                    nc.vector.bn_stats(st[:, 6:12], src_tiles[t][:, 384:768])
                    ag = p2.tile([128, 2], F32, tag="bnag", name="bnag")
                    nc.vector.bn_aggr(ag[:], st[:])
                    nmean = p2.tile([128, 1], F32, tag="nmean", name="nmean")
                    nc.scalar.mul(nmean[:], ag[:, 0:1], -1.0)
                    # rstd = exp(-0.5*ln(var+eps)): stays in the ln/exp
                    # activation table set (no table switch vs Sqrt)
                    lnv = p2.tile([128, 1], F32, tag="lnv", name="lnv")
                    nc.scalar.activation(lnv[:], ag[:, 1:2], AF.Ln, bias=epst[:])
                    rstd = p2.tile([128, 1], F32, tag="rstd", name="rstd")
                    nc.scalar.activation(rstd[:], lnv[:], AF.Exp, scale=-0.5)
                    hn = p2.tile([128, D], F32, tag="hnorm", name="hnorm")
                    nc.vector.tensor_scalar(hn[:], src_tiles[t][:], nmean[:], rstd[:],
                                            OP.add, OP.mult)
                    for d in range(KT):
                        pst = psmm.tile([128, 128], F32, tag="mm", name="mm")
                        nc.tensor.transpose(pst[:], hn[:, d * 128:(d + 1) * 128],
                                            ident[:])
                        nc.vector.tensor_copy(dst_fn(d, t), pst[:])

            for l in range(l_run):
                # ================= attention =================
                hT = [p1.tile([128, T], DT_H, tag=f"hT{k}", name=f"hT{k}") for k in range(KT)]
                layernorm_to(x, lambda d, t: hT[d][:, t * 128:(t + 1) * 128])

                kv_in_k = dram.tile([KELEM], DT_A, tag="kv_in_k", name="kv_in_k")
                kv_out_k = dram.tile([2, KELEM], DT_A, tag="kv_out_k", name="kv_out_k")
                kv_in_v = dram.tile([VELEM], DT_A, tag="kv_in_v", name="kv_in_v")
                kv_out_v = dram.tile([2, VELEM], DT_A, tag="kv_out_v", name="kv_out_v")
                kv_in_k2 = kv_in_k[:].rearrange("(r c) -> r c", c=T)
                kv_in_v2 = kv_in_v[:].rearrange("(r c) -> r c", c=VW)

                if use_bqk:
                    bqkt = p2.tile([1, QK_PAD + KP], DT_H, tag="bqk_s", name="bqk_s")
                    nc.sync.dma_start(bqkt[:], bqk_d.ap()[l])

                # K projection (packed feature-major), staged to DRAM bounce
                for mp in range(3):
                    wt = p2.tile([128, KT, 256], DT_H, tag="wqk_s", name="wqk_s")
                    nc.sync.dma_start(
                        wt[:], wqk_d.ap()[l][:, QK_PAD + mp * 256:QK_PAD + (mp + 1) * 256]
                        .rearrange("(o p) n -> p o n", p=128))
                    for mm_ in range(2):
                        m = 2 * mp + mm_
                        ps = psmm.tile([128, T], F32, tag="mm", name="mm")
                        for k in range(KT):
                            nc.tensor.matmul(ps[:], wt[:, k, mm_ * 128:(mm_ + 1) * 128],
                                             hT[k][:], start=(k == 0),
                                             stop=(k == KT - 1 and not use_bqk))
                        if use_bqk:
                            nc.tensor.matmul(
                                ps[:], bqkt[0:1, QK_PAD + m * 128:QK_PAD + (m + 1) * 128],
                                ones_row[:], start=False, stop=True)
                        kst = p2.tile([128, T], DT_A, tag="k_stage", name="k_stage")
                        nc.vector.tensor_copy(kst[:], ps[:])
                        nc.sync.dma_start(kv_in_k2[m * 128:(m + 1) * 128, :], kst[:])

                if os.environ.get("FAKE_AG"):
                    for c in range(2):
                        nc.sync.dma_start(kv_out_k[c, :], kv_in_k[:])
                else:
                    nc.gpsimd.collective_compute(
                        "AllGather", OP.bypass,
                        replica_groups=[[0, 1], [2, 3], [4, 5], [6, 7]],
                        ins=[kv_in_k[:].opt()],
                        outs=[kv_out_k[:].opt()],
                    )

                # V projection (token-major with ones-aug cols), staged
                bvt = p1.tile([1, VW], DT_H, tag="bv_s", name="bv_s")
                nc.sync.dma_start(bvt[:], bv_d.ap()[l])
                psv = [[psmm.tile([128, nlen], F32, tag="mm", name="mm")
                        for (n0, nlen) in ((0, 512), (512, VW - 512))]
                       for t in range(TT)]
                for k in range(KT):
                    wvt = p2.tile([128, VW], DT_H, tag="wv_s", name="wv_s")
                    nc.sync.dma_start(wvt[:],
                                      wv_d.ap()[l][k * 128:(k + 1) * 128, :])
                    for t in range(TT):
                        for ni, (n0, nlen) in enumerate(((0, 512), (512, VW - 512))):
                            nc.tensor.matmul(psv[t][ni][:],
                                             hT[k][:, t * 128:(t + 1) * 128],
                                             wvt[:, n0:n0 + nlen],
                                             start=(k == 0), stop=False)
                for t in range(TT):
                    vst = p2.tile([128, VW], DT_A, tag="v_stage", name="v_stage")
                    for ni, (n0, nlen) in enumerate(((0, 512), (512, VW - 512))):
                        nc.tensor.matmul(psv[t][ni][:], ones_h[:],
                                         bvt[0:1, n0:n0 + nlen], start=False, stop=True)
                        nc.vector.tensor_copy(vst[:, n0:n0 + nlen], psv[t][ni][:])
                    nc.sync.dma_start(kv_in_v2[t * 128:(t + 1) * 128, :], vst[:])

                if os.environ.get("FAKE_AG"):
                    for c in range(2):
                        nc.sync.dma_start(kv_out_v[c, :], kv_in_v[:])
                else:
                    nc.gpsimd.collective_compute(
                        "AllGather", OP.bypass,
                        replica_groups=[[0, 1], [2, 3], [4, 5], [6, 7]],
                        ins=[kv_in_v[:].opt()],
                        outs=[kv_out_v[:].opt()],
                    )

                # Q projection (padded feature-major), stays local
                for mp in range(4):
                    wt = p2.tile([128, KT, 256], DT_H, tag="wqk_s", name="wqk_s")
                    nc.sync.dma_start(
                        wt[:], wqk_d.ap()[l][:, mp * 256:(mp + 1) * 256]
                        .rearrange("(o p) n -> p o n", p=128))
                    for mm_ in range(2):
                        m = 2 * mp + mm_
                        ps = psmm.tile([128, T], F32, tag="mm", name="mm")
                        for k in range(KT):
                            nc.tensor.matmul(ps[:], wt[:, k, mm_ * 128:(mm_ + 1) * 128],
                                             hT[k][:], start=(k == 0),
                                             stop=(k == KT - 1 and not use_bqk))
                        if use_bqk:
                            nc.tensor.matmul(ps[:], bqkt[0:1, m * 128:(m + 1) * 128],
                                             ones_row[:], start=False, stop=True)
                        # evacuate head rows only, preserving q-side aug rows
                        nc.vector.tensor_copy(qT[m][0:48, :], ps[0:48, :])
                        nc.vector.tensor_copy(qT[m][64:112, :], ps[64:112, :])
                if l == 0:
                    for m in range(8):
                        nc.sync.dma_start(qT[m][48:48 + NAUG, :], qaug_d.ap()[2 * m])
                        nc.sync.dma_start(qT[m][112:112 + NAUG, :],
                                          qaug_d.ap()[2 * m + 1])

                # assemble full-sequence K (feature-major) and V (token-major);
                # packed 48-row head chunks land at 64-row offsets, skipping
                # the persistent aug rows
                for r in range(8):
                    for c in range(2):
                        nc.sync.dma_start(
                            kT_full[r][0:48, c * T:(c + 1) * T],
                            kv_out_k[c, 96 * r * T:(96 * r + 48) * T]
                            .rearrange("(p f) -> p f", p=48))
                        nc.sync.dma_start(
                            kT_full[r][64:112, c * T:(c + 1) * T],
                            kv_out_k[c, (96 * r + 48) * T:(96 * r + 96) * T]
                            .rearrange("(p f) -> p f", p=48))
                for j in range(8):
                    c, jj = j // 4, j % 4
                    nc.sync.dma_start(
                        v_full[j][:, 0:VW],
                        kv_out_v[c, jj * 128 * VW:(jj + 1) * 128 * VW]
                        .rearrange("(p f) -> p f", p=128))

                # ---- per head-pair: scores (incl aug bias) + kink bias, exp,
                # AV (col-packed).  o_pad: 8 tiles [128, T]; pair j tile holds
                # head 2j at rows 0-47 (sum at 48) and head 2j+1 at rows
                # 64-111 (sum at 112).
                o_pad = []
                sums_g = p1.tile([16, T], DT_A, tag="sums_g", name="sums_g")
                for j in range(8):
                    psav = psmm.tile([128, T], F32, tag="mm", name="mm")
                    for kt in range(8):
                        hhA, hhB = 2 * j, 2 * j + 1
                        w0 = (kt % 4) * 128
                        pssA = psmm.tile([128, T], F32, tag="mm", name="mm")
                        pssB = psmm.tile([128, T], F32, tag="mm", name="mm")
                        nc.tensor.matmul(
                            pssA[:], kT_full[j][0:64, kt * 128:(kt + 1) * 128],
                            qT[j][0:64, :], start=True, stop=False)
                        nc.tensor.matmul(
                            pssB[:], kT_full[j][64:128, kt * 128:(kt + 1) * 128],
                            qT[j][64:128, :], start=True, stop=False)
                        nc.tensor.matmul(
                            pssA[:, w0:w0 + 128], sid[:, hhA * 128:(hhA + 1) * 128],
                            kink_sb[:, kt * 128:(kt + 1) * 128],
                            start=False, stop=True)
                        nc.tensor.matmul(
                            pssB[:, w0:w0 + 128], sid[:, hhB * 128:(hhB + 1) * 128],
                            kink_sb[:, kt * 128:(kt + 1) * 128],
                            start=False, stop=True)
                        ptA = p4.tile([128, T], DT_A, tag="p", name="p")
                        nc.scalar.activation(ptA[:], pssA[:], AF.Exp)
                        ptB = p4.tile([128, T], DT_A, tag="p", name="p")
                        nc.scalar.activation(ptB[:], pssB[:], AF.Exp)
                        nc.tensor.matmul(
                            psav[0:64, :],
                            v_full[kt][:, 49 * hhA:49 * hhA + 64], ptA[:],
                            start=(kt == 0), stop=(kt == 7),
                            tile_position=(0, 0), skip_group_check=True)
                        nc.tensor.matmul(
                            psav[64:128, :],
                            v_full[kt][:, 49 * hhB:49 * hhB + 64], ptB[:],
                            start=(kt == 0), stop=(kt == 7),
                            tile_position=(0, 64), skip_group_check=True)
                    oj = p1.tile([128, T], DT_A, tag=f"oall{j}", name=f"oall{j}")
                    nc.vector.tensor_copy(oj[:], psav[:])
                    # softmax denominators sit at rows 48 / 112 (v ones-aug)
                    nc.sync.dma_start(sums_g[2 * j:2 * j + 1, :], oj[48:49, :])
                    nc.sync.dma_start(sums_g[2 * j + 1:2 * j + 2, :], oj[112:113, :])
                    o_pad.append(oj)

                rec_f = p1.tile([16, T], F32, tag="rec_f", name="rec_f")
                nc.vector.reciprocal(rec_f[:], sums_g[:])
                rec_b = p1.tile([16, T], DT_A, tag="rec_b", name="rec_b")
                nc.scalar.copy(rec_b[:], rec_f[:])

                o_scaled = []
                for j in range(8):
                    ra = p4.tile([1, T], DT_A, tag="rec_row", name="rec_row")
                    nc.sync.dma_start(ra[:], rec_b[2 * j:2 * j + 1, :])
                    rb = p4.tile([1, T], DT_A, tag="rec_row", name="rec_row")
                    nc.sync.dma_start(rb[:], rec_b[2 * j + 1:2 * j + 2, :])
                    rep = psmm.tile([128, T], F32, tag="mm", name="mm")
                    nc.tensor.matmul(rep[0:64, :], mask48[:], ra[:],
                                     start=True, stop=True, tile_position=(0, 0))
                    nc.tensor.matmul(rep[64:128, :], mask48[:], rb[:],
                                     start=True, stop=True, tile_position=(0, 64))
                    rep_sb = p2.tile([128, T], DT_A, tag="rep_sb", name="rep_sb")
                    nc.scalar.copy(rep_sb[:], rep[:])
                    osj = p1.tile([128, T], DT_A, tag=f"opad{j}", name=f"opad{j}")
                    nc.vector.tensor_tensor(osj[:], o_pad[j][:], rep_sb[:], OP.mult)
                    o_scaled.append(osj)

                # ---- Wo (pad rows carry zero weights) + residual ----
                psw = [[psmm.tile([128, nlen], F32, tag="mm", name="mm")
                        for (n0, nlen) in ((0, 512), (512, 256))]
                       for t in range(TT)]
                for k in range(8):
                    wot = p3.tile([128, D], DT_A, tag="wo_s", name="wo_s")
                    nc.sync.dma_start(wot[:],
                                      wo_d.ap()[l][k * 128:(k + 1) * 128, :])
                    for t in range(TT):
                        for ni, (n0, nlen) in enumerate(((0, 512), (512, 256))):
                            nc.tensor.matmul(psw[t][ni][:],
                                             o_scaled[k][:, t * 128:(t + 1) * 128],
                                             wot[:, n0:n0 + nlen], start=(k == 0),
                                             stop=(k == 7))
                for t in range(TT):
                    for ni, (n0, nlen) in enumerate(((0, 512), (512, 256))):
                        nc.vector.tensor_tensor(x[t][:, n0:n0 + nlen],
                                                x[t][:, n0:n0 + nlen],
                                                psw[t][ni][:], OP.add)

                # ================= SwiGLU FFN =================
                h2T = p1.tile([128, KT, T], DT_H, tag="h2T", name="h2T")
                layernorm_to(x, lambda d, t: h2T[:, d, t * 128:(t + 1) * 128])

                h3 = []
                for f in range(FT):
                    wgu = p2.tile([128, KT, 256], DT_H, tag="wgu_s", name="wgu_s")
                    nc.sync.dma_start(
                        wgu[:], wgu_d.ap()[l][:, f * 256:(f + 1) * 256]
                        .rearrange("(o p) n -> p o n", p=128))
                    psg = psmm.tile([128, T], F32, tag="mm", name="mm")
                    for k in range(KT):
                        nc.tensor.matmul(psg[:], wgu[:, k, 0:128],
                                         h2T[:, k, :], start=(k == 0),
                                         stop=(k == KT - 1))
                    gsb = p2.tile([128, T], DT_A, tag="g_sb", name="g_sb")
                    nc.scalar.activation(gsb[:], psg[:], AF.Silu)
                    psu = psmm.tile([128, T], F32, tag="mm", name="mm")
                    for k in range(KT):
                        nc.tensor.matmul(psu[:], wgu[:, k, 128:256],
                                         h2T[:, k, :], start=(k == 0),
                                         stop=(k == KT - 1))
                    h3f = p1.tile([128, T], DT_A, tag=f"h3_{f}", name=f"h3_{f}")
                    nc.vector.tensor_tensor(h3f[:], gsb[:], psu[:], OP.mult)
                    h3.append(h3f)

                psd = [[psmm.tile([128, nlen], F32, tag="mm", name="mm")
                        for (n0, nlen) in ((0, 512), (512, 256))] for t in range(TT)]
                for f in range(FT):
                    wdt = p3.tile([128, D], DT_A, tag="wd_s", name="wd_s")
                    nc.sync.dma_start(wdt[:], wd_d.ap()[l][f * 128:(f + 1) * 128, :])
                    for t in range(TT):
                        for ni, (n0, nlen) in enumerate(((0, 512), (512, 256))):
                            nc.tensor.matmul(psd[t][ni][:],
                                             h3[f][:, t * 128:(t + 1) * 128],
                                             wdt[:, n0:n0 + nlen],
                                             start=(f == 0), stop=(f == FT - 1))
                for t in range(TT):
                    for ni, (n0, nlen) in enumerate(((0, 512), (512, 256))):
                        nc.vector.tensor_tensor(x[t][:, n0:n0 + nlen],
                                                x[t][:, n0:n0 + nlen],
                                                psd[t][ni][:], OP.add)

            # ---- final layernorm of the CLS row (token 0) + affine ----
            finw = p1.tile([1, D], F32, tag="finw", name="finw")
            nc.sync.dma_start(finw[:], finw_d.ap())
            finb = p1.tile([1, D], F32, tag="finb", name="finb")
            nc.sync.dma_start(finb[:], finb_d.ap())

            x0r = x[0][0:1, :]
            fst = p2.tile([1, 12], F32, tag="fbnst", name="fbnst")
            nc.vector.bn_stats(fst[:, 0:6], x0r[:, 0:384])
            nc.vector.bn_stats(fst[:, 6:12], x0r[:, 384:768])
            fag = p2.tile([1, 2], F32, tag="fbnag", name="fbnag")
            nc.vector.bn_aggr(fag[:], fst[:])
            nmean = p2.tile([1, 1], F32, tag="fnmean", name="fnmean")
            nc.scalar.mul(nmean[:], fag[:, 0:1], -1.0)
            xc = p1.tile([1, D], F32, tag="fxc", name="fxc")
            nc.vector.tensor_scalar(xc[:], x0r, nmean[:], None, OP.add)
            flnv = p2.tile([1, 1], F32, tag="flnv", name="flnv")
            nc.scalar.activation(flnv[:], fag[:, 1:2], AF.Ln, bias=epst[0:1, :])
            rstd = p2.tile([1, 1], F32, tag="frstd", name="frstd")
            nc.scalar.activation(rstd[:], flnv[:], AF.Exp, scale=-0.5)
            yt = p1.tile([1, D], F32, tag="fy", name="fy")
            nc.vector.tensor_scalar(yt[:], xc[:], rstd[:], None, OP.mult)
            nc.vector.tensor_tensor(yt[:], yt[:], finw[:], OP.mult)
            nc.vector.tensor_tensor(yt[:], yt[:], finb[:], OP.add)
            nc.sync.dma_start(y_d.ap(), yt[:])

    nc.compile()
    return nc


def _alibi_aug(slopes_np, half):
    """Per-core ALiBi augmentation tensors.

    Returns kaug [H, NAUG, S], qaug [H, NAUG, T], kink [128, 8*128] (all
    float64; caller casts).  half=0: queries 0..511, half=1: queries 512..1023.

    Aug pairs p (rows 2p, 2p+1) satisfy, for key i / global query j:
      sum_p kaug[h,2p,i]*qaug[h,2p,j] + kaug[h,2p+1,i]*qaug[h,2p+1,j]
        = -slope_h * |i-j|
    for all (i, j) outside the per-key-block 128-query kink windows; the
    kink windows are covered by the slope-diagonal matmul over `kink`.
    """
    q0 = half * T
    jj = q0 + np.arange(T, dtype=np.float64)       # global query positions
    ii = np.arange(S, dtype=np.float64)            # global key positions
    kaug = np.zeros((H, NAUG, S), np.float64)
    qaug = np.zeros((H, NAUG, T), np.float64)

    # straddle blocks: the 4 key blocks overlapping the query window
    sb = [half * 4 + b for b in range(4)]

    # pair list: (key_mask, q_mask, sign) with sign = +1 when key is above
    # query (i > j, bias = -s*(i-j)), -1 when below (bias = -s*(j-i)).
    pairs = []
    if half == 0:
        # far blocks 4..7 (keys above queries): near block 4, rest 5-7
        pairs.append((np.logical_and(ii >= 512, ii < 640), np.ones(T, bool), +1, 576.0))
        pairs.append((ii >= 640, np.ones(T, bool), +1, 832.0))
    else:
        # far blocks 0..3 (keys below queries): near block 3, rest 0-2
        pairs.append((np.logical_and(ii >= 384, ii < 512), np.ones(T, bool), -1, 448.0))
        pairs.append((ii < 384, np.ones(T, bool), -1, 192.0))
    # straddle A-pairs: key block sb[b], queries strictly before the block
    for b in (1, 2, 3):
        blk = sb[b]
        km = np.logical_and(ii >= 128 * blk, ii < 128 * (blk + 1))
        qm = (jj < 128 * blk)
        pairs.append((km, qm, +1, 128 * blk + 64.0))
    # straddle B-pairs: key block sb[b], queries strictly after the block
    for b in (0, 1, 2):
        blk = sb[b]
        km = np.logical_and(ii >= 128 * blk, ii < 128 * (blk + 1))
        qm = (jj >= 128 * (blk + 1))
        pairs.append((km, qm, -1, 128 * blk + 64.0))

    for h in range(H):
        s = slopes_np[h]
        for p, (km, qm, sign, c) in enumerate(pairs):
            # bias = -s*sign*(i-j) = [-s*sign*(i-c)]*1 + [s*sign]*(j-c)
            kaug[h, 2 * p, :] = -s * sign * (ii - c) * km
            qaug[h, 2 * p, :] = qm.astype(np.float64)
            kaug[h, 2 * p + 1, :] = s * sign * km
            qaug[h, 2 * p + 1, :] = (jj - c) * qm

    kink = np.zeros((128, 8 * 128), np.float64)
    for b in range(4):
        blk = sb[b]
        keys = 128 * blk + np.arange(128, dtype=np.float64)
        qwin = q0 + 128 * b + np.arange(128, dtype=np.float64)
        kink[:, blk * 128:(blk + 1) * 128] = np.abs(
            keys[:, None] - qwin[None, :])
    return kaug, qaug, kink


def prepare_inputs(cls_tokens, cls_token, log_slopes, Wqkv, Wo, Wg, Wu, Wd,
                   ln1_w, ln1_b, ln2_w, ln2_b, fin_w, fin_b):
    """Fold LN affine params into weights, pad heads, build per-core arrays."""
    f32 = np.float32
    scale = 1.0 / math.sqrt(HD)

    slopes_np = np.exp(np.asarray(log_slopes, np.float64))
    wqk = np.zeros((L, D, QK_PAD + KP), f32)
    bqk = np.zeros((L, 1, QK_PAD + KP), f32)
    wv = np.zeros((L, D, VW), f32)
    bv = np.zeros((L, 1, VW), f32)
    wo = np.zeros((L, QK_PAD, D), f32)
    wgu = np.zeros((L, D, 2 * FF), f32)
    bg = np.zeros((L, 1, FF), f32)
    bu = np.zeros((L, 1, FF), f32)
    wd = np.zeros((L, FF, D), f32)

    for l in range(L):
        W1 = (np.asarray(Wqkv[l], np.float64) *
              np.asarray(ln1_w[l], np.float64)[None, :])
        b1 = np.asarray(Wqkv[l], np.float64) @ np.asarray(ln1_b[l], np.float64)
        for h in range(H):
            qs = slice(48 * h, 48 * h + 48)
            wqk[l, :, 64 * h:64 * h + 48] = (W1[qs].T * scale)
            bqk[l, 0, 64 * h:64 * h + 48] = b1[qs] * scale
            ks = slice(D + 48 * h, D + 48 * h + 48)
            wqk[l, :, QK_PAD + 48 * h:QK_PAD + 48 * h + 48] = W1[ks].T
            bqk[l, 0, QK_PAD + 48 * h:QK_PAD + 48 * h + 48] = b1[ks]
            vs = slice(2 * D + 48 * h, 2 * D + 48 * h + 48)
            wv[l, :, 49 * h:49 * h + 48] = W1[vs].T
            bv[l, 0, 49 * h:49 * h + 48] = b1[vs]
            bv[l, 0, 49 * h + 48] = 1.0
        woT = np.asarray(Wo[l], f32).T
        for h in range(H):
            wo[l, 64 * h:64 * h + 48, :] = woT[48 * h:48 * h + 48, :]
        W2g = (np.asarray(Wg[l], np.float64) *
               np.asarray(ln2_w[l], np.float64)[None, :])
        W2u = (np.asarray(Wu[l], np.float64) *
               np.asarray(ln2_w[l], np.float64)[None, :])
        wgu_l = wgu[l].reshape(D, FT, 2, 128)
        wgu_l[:, :, 0, :] = W2g.T.reshape(D, FT, 128)
        wgu_l[:, :, 1, :] = W2u.T.reshape(D, FT, 128)
        bg[l, 0] = np.asarray(Wg[l], np.float64) @ np.asarray(ln2_b[l], np.float64)
        bu[l, 0] = np.asarray(Wu[l], np.float64) @ np.asarray(ln2_b[l], np.float64)
        wd[l] = np.asarray(Wd[l], f32).T

    use_bqk = bool(np.any(bqk != 0))
    use_bgu = bool(np.any(bg != 0) or np.any(bu != 0))

    sid = np.zeros((128, H * 128), np.float16)
    for h in range(H):
        sid[:, h * 128:(h + 1) * 128] = -slopes_np[h] * np.eye(128)

    x_full = np.concatenate(
        [np.broadcast_to(np.asarray(cls_token, f32), (B, 1, D)),
         np.asarray(cls_tokens, f32)], axis=1)  # (B, S, D)

    np_a = mybir.dt.np(DT_A)
    np_h = mybir.dt.np(DT_H)
    common = {
        "wqk": wqk.astype(np_h), "wv": wv.astype(np_h), "bv": bv.astype(np_h),
        "wo": wo.astype(np_a), "wgu": wgu.astype(np_h),
        "wd": wd.astype(np_a), "sid": sid,
        "finw": np.asarray(fin_w, f32).reshape(1, D),
        "finb": np.asarray(fin_b, f32).reshape(1, D),
    }
    if use_bqk:
        common["bqk"] = bqk.astype(np_h)
    if use_bgu:
        common["bg"] = bg.astype(np_h)
        common["bu"] = bu.astype(np_h)

    aug_cache = {}
    in_maps = []
    for c in range(NCORES):
        b, half = c // 2, c % 2
        q0 = T * half
        if half not in aug_cache:
            kaug, qaug, kink = _alibi_aug(slopes_np, half)
            aug_cache[half] = (kaug.astype(np_a), qaug.astype(np_a),
                              kink.astype(np.float16))
        kaug_a, qaug_a, kink_a = aug_cache[half]
        m = dict(common)
        m["x0"] = np.ascontiguousarray(x_full[b, q0:q0 + T])
        m["kaug"] = kaug_a
        m["qaug"] = qaug_a
        m["kinkd"] = kink_a
        in_maps.append(m)
    return in_maps, use_bqk, use_bgu


def kernel(**inputs):
    in_maps, use_bqk, use_bgu = prepare_inputs(**inputs)
    slopes = np.exp(np.asarray(inputs["log_slopes"], np.float64))
    key = (use_bqk, use_bgu, tuple(np.round(slopes, 10)))
    if key not in _NC_CACHE:
        _NC_CACHE[key] = build_nc(use_bqk, use_bgu, slopes=slopes)
    nc = _NC_CACHE[key]
    res = run_bass_kernel_spmd(nc, in_maps, core_ids=list(range(NCORES)))
    out = np.stack([res.results[2 * b]["y"][0] for b in range(B)])
    return out.astype(np.float32)
